# revision 25
# baseline (speedup 1.0000x reference)
"""STCN/STM-style memory read (retrieval_knn) on 8 Trainium2 NeuronCores.

Reference computation (per batch b):
    mk  [64, 8000]  memory keys     (THW = 5*40*40 = 8000)
    mv  [512, 8000] memory values
    qk  [64, 1600]  query keys      (HW = 1600)
    sim = (2 * mk.T @ qk - ||mk||^2) / 8          # [8000, 1600]
    attn = softmax(sim, axis=0)
    out = mv @ attn                                # [512, 1600]

Sharding: 8 cores = 4 batches x 2 query-halves; each core handles 800
query pixels in two chunks (448, 352) and 64 memory tiles of 128 rows
(8000 padded to 8192).

Mixed-precision scheme, built around fp8-e4m3 DoubleRow matmuls (two
128-row k-tile slots contracted per instruction at 0.5 cycles/output
column = 4x fp16 column throughput):

  sim (all tiles): ONE DoubleRow matmul per tile; the spare K rows carry
             the qk-lo correction products:
               slot0 = [mk_hi | mksq c1,c2,c3 | mk_hi[0:61]]
                     x [qk_hi |     -0.5x3    | qk_lo[0:61]]
               slot1 = [mk_lo | mk_hi[61:64] | 1 | 0]
                     x [qk_hi | qk_lo[61:64] | 5.5 | 0]
             => full (hi+lo)x(hi+lo) minus the negligible lo*lo term, i.e.
             near-fp16 accuracy at half the cost. The 1 x 5.5 row adds a
             +1.375 bias to sim so e = 3.96*exp(sim): protects the e4m3
             denormal tail and cancels in the final division. e4m3 max is
             240; weights peak ~165.
  exp:       ScalarE Exp(0.25*psum) per tile pair; hot -> fp16, cold e4m3.
  readout:   memory rows are sorted by ||mk||^2 ascending (host-side, per
             batch), three tiers by softmax mass: tiles 0-11 ("hot") read
             out in fp16; tiles 12-39 ("mid") use 2 DoubleRow matmuls per
             tile-pair per cv tile (mv_hi-pair x e8-pair + mv_lo-pair x
             e8-pair; the hi/lo split removes mv quantization error);
             tiles 40-63 ("far", ~3% of squared mass) drop the mv_lo
             product. Total output rel err 1.74e-2 vs the 2e-2 gate.
  denominator: cold = ones x e8 DoubleRow matmuls into a [16, q] psum
             (dual-fp8 ldweights need stationary free >= 16), emitted as a
             sweep near the loop end where the sim psum pool has a free
             slot; hot = DVE fp32 accumulation of e16, folded in with one
             fp16 ones-column matmul.
  finish:    DVE reciprocal -> fp16, broadcast via ones-row matmul, DVE
             multiply -> fp16 staging, one DMA per chunk; host casts the
             fp16 output to fp32.

All matmuls accumulate into 4 cv psum tiles in a single pass (PSUM: 2x2
banks sim pairs + 4 banks out). Emission is software-pipelined (next
pair's sims emitted between a pair's exp and its readouts). Key/value
DMAs are batched (4-8 tiles per copy) and ordered so each piece lands
just before its consumers (~625ns HWDGE cost per DMA instruction).

The kernel is PE-bound and largely PE-instruction-count-bound: ~560
matmuls, each a Ldweights+Matmult pair with ~30ns sequencer/queue
friction — which is why cold tiles use few fused products, not more
smaller ops. TimelineSim: 89.9us vs 125.7us for the fp16 baseline
(externally measured at 114.8us).
"""

import sys

sys.path.insert(0, "/opt/trn_rl_repo")

import math

import numpy as np
import ml_dtypes

B, CK, CV, T, H, W = 4, 64, 512, 5, 40, 40
THW = T * H * W          # 8000
HW = H * W               # 1600
NT = 64                  # memory tiles after padding (8192 rows)
MPAD = NT * 128          # 8192
NH = 12                  # hot (fp16-readout) tiles
FAR_B = 40               # tiles beyond this use a single-product readout
NC = NT - NH             # cold (fp8) tiles
NP = NC // 2             # cold tile pairs
KDIM = 128
NCORES = 8
Q = HW // 2              # 800 query pixels per core
CHUNKS = (448, 352)
NCV = CV // 128          # 4
# exp scale: e = exp(sim + 1.375) = 3.96*exp(sim); cancels in num/den. The
# offset rides a spare aug row (mk-side 1.0 x qk-side 5.5, exact in fp16 and
# e4m3) so hot and cold tiers apply bit-identical scaling.
SCALE_ROW = 5.5
PAD_MKSQ = 240.0         # pad-row ||mk||^2 (e4m3-max) -> exp ~ e-30 -> 0

F8 = ml_dtypes.float8_e4m3

_CACHE = {}
LAST_RESULTS = None      # BassKernelResults of the most recent run (for test.py)


def _build_program(n_reps=1):
    import concourse.bacc as bacc
    import concourse.bass as bass
    import concourse.mybir as mybir
    import concourse.tile as tile
    from concourse.bass import ts

    f8 = mybir.dt.float8e4
    f16 = mybir.dt.float16
    f32 = mybir.dt.float32
    Exp = mybir.ActivationFunctionType.Exp
    DR = mybir.MatmulPerfMode.DoubleRow

    nc = bacc.Bacc(None, target_bir_lowering=False)

    mkp8_d = nc.dram_tensor("mkp8", [KDIM, 2, NT * 128], f8, kind="ExternalInput")
    qkp8_d = nc.dram_tensor("qkp8", [KDIM, 2, Q], f8, kind="ExternalInput")
    # values arrive pre-grouped for batched DMA (4 tiles / 4 pairs per copy)
    mv16_d = nc.dram_tensor("mv16", [128, NH // 4, 4, CV], f16, kind="ExternalInput")
    NG8 = (NC + 7) // 8      # mv8 groups (last one may be padded)
    mvh8_d = nc.dram_tensor("mvh8", [128, NG8, 8, CV], f8, kind="ExternalInput")
    mvl8_d = nc.dram_tensor("mvl8", [128, NG8, 8, CV], f8, kind="ExternalInput")
    out_d = nc.dram_tensor("out", [128, NCV, Q], f16, kind="ExternalOutput")

    with tile.TileContext(nc) as tc:
        with (
            tc.tile_pool(name="const", bufs=1) as cpool,
            tc.tile_pool(name="keys", bufs=1) as kpool,
            tc.tile_pool(name="mv16", bufs=NH // 4 + 1) as mv16pool,
            tc.tile_pool(name="mv8", bufs=(NC + 7) // 8 + 1) as mv8pool,
            tc.tile_pool(name="work", bufs=2) as wpool,
            tc.tile_pool(name="e16", bufs=NH // 2 + 2) as e16pool,
            tc.tile_pool(name="e8", bufs=NP + 3) as e8pool,
            tc.tile_pool(name="osb", bufs=2) as opool,
            tc.tile_pool(name="ps_out", bufs=4, space="PSUM") as ps_out,
            tc.tile_pool(name="ps_sim", bufs=2, space="PSUM") as ps_sim,
        ):
            # dual-fp8 ldweights require stationary free size >= 16, so the
            # denominator rides a 16-row ones block (row 0 is used downstream)
            ones8 = cpool.tile([128, 2, 16], f8, name="ones8")
            nc.vector.memset(ones8[:], 1.0)
            ones_col16 = cpool.tile([128, 1], f16, name="ones_col16")
            nc.vector.memset(ones_col16[:], 1.0)
            ones_row = cpool.tile([1, 128], f16, name="ones_row")
            nc.vector.memset(ones_row[:], 1.0)

            import contextlib

            loop_ctx = (
                tc.For_i(0, n_reps, 1, hint_engines=(mybir.EngineType.PE,))
                if n_reps > 1
                else contextlib.nullcontext()
            )
            with loop_ctx:
                r = "r0_"
                # sim inputs first so the PE can start ~2.5us in; keys in 3
                # pieces (hot tiles, then cold halves) so early sims don't
                # wait on the tail of the 2MB key transfer
                qkp8_s = kpool.tile([KDIM, 2, Q], f8, name=r + "qkp8", tag="qk8")
                nc.sync.dma_start(
                    qkp8_s[:, :, : CHUNKS[0]], qkp8_d[:, :, : CHUNKS[0]]
                )
                # key pieces sized so each arrives just before its consumers:
                # 4 hot tiles (first sims), rest of hot, two cold halves.
                # mv16 groups interleave so hot readouts aren't DMA-gated.
                MK_SPLITS = (0, 4 * 128, NH * 128, (NH + NC // 2) * 128, NT * 128)
                mkp8_parts = []

                def load_mk(j):
                    lo, hi = MK_SPLITS[j], MK_SPLITS[j + 1]
                    p = kpool.tile(
                        [KDIM, 2, hi - lo], f8, name=f"{r}mkp8_{j}", tag=f"mk8{j}"
                    )
                    nc.sync.dma_start(p[:], mkp8_d[:, :, bass.ds(lo, hi - lo)])
                    mkp8_parts.append(p)

                mv16_grps = []

                def load_mv16(g):
                    mg = mv16pool.tile([128, 4, CV], f16, name=f"{r}mv16_{g}", tag="mv16")
                    nc.sync.dma_start(mg[:], mv16_d[:, g, :, :])
                    mv16_grps.append(mg)

                load_mk(0)
                nc.sync.dma_start(
                    qkp8_s[:, :, CHUNKS[0] :], qkp8_d[:, :, CHUNKS[0] :]
                )
                load_mv16(0)
                load_mk(1)
                load_mv16(1)
                load_mv16(2)
                load_mk(2)
                load_mk(3)
                mvh_grps, mvl_grps = [], []
                NEED_MVL = (FAR_B - NH + 7) // 8     # mv-lo only below FAR_B
                for g in range((NC + 7) // 8):
                    th = mv8pool.tile([128, 8, CV], f8, name=f"{r}mvh_{g}", tag="mvh")
                    nc.sync.dma_start(th[:], mvh8_d[:, g, :, :])
                    mvh_grps.append(th)
                    if g < NEED_MVL:
                        tl = mv8pool.tile(
                            [128, 8, CV], f8, name=f"{r}mvl_{g}", tag="mvl"
                        )
                        nc.sync.dma_start(tl[:], mvl8_d[:, g, :, :])
                        mvl_grps.append(tl)

                def mv16_lhsT(t, cv):
                    return mv16_grps[t // 4][:, t % 4, ts(cv, 128)]

                def mvh_lhsT(p, cv):
                    return mvh_grps[p // 4][:, bass.ds(2 * (p % 4), 2), ts(cv, 128)]

                def mvl_lhsT(p, cv):
                    return mvl_grps[p // 4][:, bass.ds(2 * (p % 4), 2), ts(cv, 128)]

                def mkp8_lhsT(t):
                    # global tile index -> [128, 2, 128] slot-packed lhsT
                    col = t * 128
                    for j in range(len(MK_SPLITS) - 1):
                        if col < MK_SPLITS[j + 1]:
                            return mkp8_parts[j][
                                :, :, bass.ds(col - MK_SPLITS[j], 128)
                            ]
                    raise AssertionError(t)

                qoff = 0
                for qc, CSZ in enumerate(CHUNKS):
                    qsl = bass.ds(qoff, CSZ)

                    acc32 = wpool.tile([128, CSZ], f32, name=f"{r}acc{qc}", tag="acc")
                    acc16 = wpool.tile([128, CSZ], f16, name=f"{r}ac16{qc}", tag="ac16")

                    e16_tiles = []   # one [128,2,CSZ] f16 per hot pair
                    e8_tiles = []    # one [128,2,CSZ] f8 per cold pair

                    # Single pass over all 4 cv tiles; the denominator is
                    # accumulated afterwards in a short sweep over the
                    # retained e tiles (its psum tile reuses the sim pool,
                    # which is idle by then). Emission is software-pipelined:
                    # the NEXT pair's sim matmuls are emitted between a
                    # pair's exp and its readouts so the PE never
                    # head-of-line blocks on the ScalarE exp latency.
                    outs = [
                        ps_out.tile([128, CSZ], f32, name=f"{r}o{qc}_{cv}", tag="out")
                        for cv in range(NCV)
                    ]

                    pairs = [("h", p) for p in range(NH // 2)] + [
                        ("c", p) for p in range(NP)
                    ]

                    def emit_sim(kind, p):
                        simp = ps_sim.tile(
                            [128, 2, 512], f32, name=f"{r}s{kind}{qc}_{p}", tag="sim"
                        )
                        base = 2 * p if kind == "h" else NH + 2 * p
                        for i in range(2):
                            nc.tensor.matmul(
                                simp[:, i, :CSZ],
                                mkp8_lhsT(base + i),
                                qkp8_s[:, :, qsl],
                                start=True,
                                stop=True,
                                perf_mode=DR,
                            )
                        return simp

                    simp = emit_sim(*pairs[0])
                    den_ps = None
                    den_done = 0
                    for idx, (kind, p) in enumerate(pairs):
                        cur = simp
                        if kind == "h":
                            e16 = e16pool.tile(
                                [128, 2, CSZ], f16, name=f"{r}e16_{qc}_{p}", tag="e16"
                            )
                            nc.scalar.activation(
                                e16[:], cur[:, :, :CSZ], Exp, scale=0.25
                            )
                            e16_tiles.append(e16)
                        else:
                            e8 = e8pool.tile(
                                [128, 2, CSZ], f8, name=f"{r}e8_{qc}_{p}", tag="e8"
                            )
                            nc.scalar.activation(
                                e8[:], cur[:, :, :CSZ], Exp, scale=0.25
                            )
                            e8_tiles.append(e8)
                        if idx + 1 < len(pairs):
                            simp = emit_sim(*pairs[idx + 1])
                        if idx >= len(pairs) - 2 and den_ps is None:
                            # last pair: start the denominator sweep over the
                            # e8 tiles already produced, overlapping the tail
                            # readouts. Its psum slot comes from the sim pool.
                            den_ps = ps_sim.tile(
                                [16, CSZ], f32, name=f"{r}den{qc}", tag="sim"
                            )
                            for cp, e8t in enumerate(e8_tiles):
                                nc.tensor.matmul(
                                    den_ps[:],
                                    ones8[:],
                                    e8t[:],
                                    start=(cp == 0),
                                    stop=False,
                                    perf_mode=DR,
                                    skip_group_check=True,
                                )
                                den_done = cp + 1
                        if kind == "h":
                            # hot denominator: DVE fp32 accumulation
                            if p == 0:
                                nc.vector.tensor_copy(acc32[:], e16[:, 0, :])
                            else:
                                nc.vector.tensor_add(acc32[:], acc32[:], e16[:, 0, :])
                            nc.vector.tensor_add(acc32[:], acc32[:], e16[:, 1, :])
                            if p == NH // 2 - 1:
                                # fp16 copy so the fold matmul runs at fp16
                                # speed (values O(1e3), fp16 rel 5e-4: fine)
                                nc.vector.tensor_copy(acc16[:], acc32[:])
                            for i in range(2):
                                t = 2 * p + i
                                for cv in range(NCV):
                                    nc.tensor.matmul(
                                        outs[cv][:],
                                        mv16_lhsT(t, cv),
                                        e16[:, i, :],
                                        start=(t == 0),
                                        stop=False,
                                        skip_group_check=True,
                                    )
                        else:
                            far = NH + 2 * p >= FAR_B
                            for cv in range(NCV):
                                last = p == NP - 1 and cv == NCV - 1
                                nc.tensor.matmul(
                                    outs[cv][:],
                                    mvh_lhsT(p, cv),
                                    e8[:],
                                    start=False,
                                    stop=(last and far),
                                    perf_mode=DR,
                                    skip_group_check=True,
                                )
                                if not far:
                                    nc.tensor.matmul(
                                        outs[cv][:],
                                        mvl_lhsT(p, cv),
                                        e8[:],
                                        start=False,
                                        stop=last,
                                        perf_mode=DR,
                                        skip_group_check=True,
                                    )

                    # ---------- denominator: remaining cold tiles
                    for cp in range(den_done, len(e8_tiles)):
                        nc.tensor.matmul(
                            den_ps[:],
                            ones8[:],
                            e8_tiles[cp][:],
                            start=(cp == 0),
                            stop=False,
                            perf_mode=DR,
                            skip_group_check=True,
                        )
                    # fold the hot-tile DVE accumulation (fp16 matmul)
                    nc.tensor.matmul(
                        den_ps[bass.ds(0, 1), :],
                        ones_col16[:],
                        acc16[:],
                        start=False,
                        stop=True,
                        skip_group_check=True,
                    )

                    recip = wpool.tile([1, CSZ], f16, name=f"{r}rcp{qc}", tag="rcp")
                    with nc.allow_low_precision(
                        reason="fp16 reciprocal of O(100) denominator: 5e-4 rel"
                    ):
                        nc.vector.reciprocal(recip[:], den_ps[bass.ds(0, 1), :])
                    bc = ps_sim.tile([128, 512], f32, name=f"{r}bc{qc}", tag="sim")
                    nc.tensor.matmul(
                        bc[:, :CSZ], ones_row[:], recip[:], start=True, stop=True
                    )
                    bc_sb = wpool.tile([128, CSZ], f32, name=f"{r}bcs{qc}", tag="bcs")
                    nc.scalar.copy(bc_sb[:], bc[:, :CSZ])
                    o_sb = opool.tile(
                        [128, NCV, CSZ], f16, name=f"{r}os{qc}", tag="osb"
                    )
                    for cv in range(2):
                        nc.vector.tensor_mul(o_sb[:, cv, :], outs[cv][:], bc_sb[:])
                    nc.sync.dma_start(out_d[:, :2, qsl], o_sb[:, :2, :])
                    for cv in range(2, NCV):
                        nc.vector.tensor_mul(o_sb[:, cv, :], outs[cv][:], bc_sb[:])
                    nc.sync.dma_start(out_d[:, 2:, qsl], o_sb[:, 2:, :])
                    qoff += CSZ

    nc.compile()
    return nc


def _get_program():
    if "nc" not in _CACHE:
        _CACHE["nc"] = _build_program()
    return _CACHE["nc"]


def _q8(x):
    return np.clip(np.asarray(x, np.float32), -240.0, 240.0).astype(F8)


def host_prep(mem_key, mem_val, qry_key):
    """Layout/sharding prep: returns per-core input maps."""
    mem_key = np.asarray(mem_key, dtype=np.float32)
    mem_val = np.asarray(mem_val, dtype=np.float32)
    qry_key = np.asarray(qry_key, dtype=np.float32)

    mk_all = mem_key.reshape(B, CK, THW)
    mv_all = mem_val.reshape(B, CV, THW)
    qk_all = qry_key.reshape(B, CK, HW)

    per_batch = []
    for b in range(B):
        mk, mv, qk = mk_all[b], mv_all[b], qk_all[b]
        mksq = np.einsum("cm,cm->m", mk, mk)
        order = np.argsort(mksq, kind="stable")
        mk = mk[:, order]
        mv = mv[:, order]
        mksq = mksq[order]

        mkp = np.zeros((CK, MPAD), np.float32)
        mkp[:, :THW] = mk
        mvp = np.zeros((CV, MPAD), np.float32)
        mvp[:, :THW] = mv
        msq = np.full(MPAD, PAD_MKSQ, np.float32)
        msq[:THW] = mksq

        nh = NH * 128
        # ---- packed fp8 keys (all tiles; hot/cold differ only downstream)
        mkc = mkp
        msqc = msq
        mh = _q8(mkc)
        ml = _q8(mkc - mh.astype(np.float32))
        c1 = _q8(msqc)
        c2 = _q8(msqc - c1.astype(np.float32))
        c3 = _q8(msqc - c1.astype(np.float32) - c2.astype(np.float32))
        qh = _q8(qk)
        ql = _q8(qk - qh.astype(np.float32))

        mkp8 = np.zeros((KDIM, 2, NT * 128), F8)
        mkp8[:CK, 0] = mh
        mkp8[CK, 0] = c1
        mkp8[CK + 1, 0] = c2
        mkp8[CK + 2, 0] = c3
        mkp8[CK + 3 :, 0] = mh[: KDIM - CK - 3]          # rows 67.. = mh[0:61]
        mkp8[:CK, 1] = ml
        mkp8[CK : CK + 3, 1] = mh[KDIM - CK - 3 : CK]    # mh[61:64]
        mkp8[CK + 3, 1] = 1.0                            # scale row

        qkp8 = np.zeros((KDIM, 2, HW), F8)
        qkp8[:CK, 0] = qh
        qkp8[CK : CK + 3, 0] = -0.5
        qkp8[CK + 3 :, 0] = ql[: KDIM - CK - 3]
        qkp8[:CK, 1] = qh
        qkp8[CK : CK + 3, 1] = ql[KDIM - CK - 3 : CK]
        qkp8[CK + 3, 1] = SCALE_ROW

        # ---- values, grouped for batched DMA
        # mv16 [128, NH//4, 4, CV]: [p, g, i, c] = mv[(4g+i)*128+p, c]
        mv16 = np.ascontiguousarray(
            mvp[:, :nh].T.astype(np.float16).reshape(NH // 4, 4, 128, CV)
            .transpose(2, 0, 1, 3)
        )
        mvc = mvp[:, nh:]                                  # [CV, NC*128]
        mvh_f = _q8(mvc)
        mvl_f = _q8(mvc - mvh_f.astype(np.float32))
        # [128, NG8, 8, CV]: [p, g, s, c] = mv8[(8g+s)*128 + p, c]
        ng8 = (NC + 7) // 8
        if ng8 * 8 != NC:
            pad = np.zeros((CV, (ng8 * 8 - NC) * 128), F8)
            mvh_f = np.concatenate([mvh_f, pad], axis=1)
            mvl_f = np.concatenate([mvl_f, pad], axis=1)
        mvh8 = np.ascontiguousarray(
            mvh_f.reshape(CV, ng8, 8, 128).transpose(3, 1, 2, 0)
        )
        mvl8 = np.ascontiguousarray(
            mvl_f.reshape(CV, ng8, 8, 128).transpose(3, 1, 2, 0)
        )
        per_batch.append((mkp8, qkp8, mv16, mvh8, mvl8))

    in_maps = []
    for c in range(NCORES):
        b, h = divmod(c, 2)
        mkp8, qkp8, mv16, mvh8, mvl8 = per_batch[b]
        sl = slice(h * Q, (h + 1) * Q)
        in_maps.append(
            {
                "mkp8": mkp8,
                "qkp8": np.ascontiguousarray(qkp8[:, :, sl]),
                "mv16": mv16,
                "mvh8": mvh8,
                "mvl8": mvl8,
            }
        )
    return in_maps


def kernel(mem_key, mem_val, qry_key):
    global LAST_RESULTS
    import os

    # this container's axon client has no NTFF hook; the trace path would
    # crash run_bass_kernel_spmd, so force it off
    os.environ["BASS_NEVER_TRACE"] = "1"
    from concourse.bass_utils import run_bass_kernel_spmd

    in_maps = host_prep(mem_key, mem_val, qry_key)
    nc = _get_program()
    LAST_RESULTS = run_bass_kernel_spmd(nc, in_maps, list(range(NCORES)))

    out = np.empty((B, CV, HW), np.float32)
    for c in range(NCORES):
        b, h = divmod(c, 2)
        o = LAST_RESULTS.results[c]["out"]          # [128, NCV, Q] fp16
        out[b, :, h * Q : (h + 1) * Q] = (
            o.astype(np.float32).transpose(1, 0, 2).reshape(CV, Q)
        )
    return out.reshape(B, CV, H, W)
